# revision 13
# baseline (speedup 1.0000x reference)
"""BERT self-attention (B=8, S=1024, D=1024, H=16, DH=64) on 8 Trainium2 cores.

Strategy: pure data-parallel over batch - each of the 8 cores runs the full
self-attention for one batch element. No collectives.

v3 layout (HW-measured evolution of the bf16 v2; fp8 was tried and
rejected - fp8 probs alone push absmax rel err to ~2.4e-2, over the 2e-2
gate, because ~3.6% per-element quantization accumulates over 1024-term
context sums and the gate is an absmax over 8M outputs):
  - bf16 datapath everywhere on the PE; Q/K biases folded into the
    PSUM->SBUF copies as per-partition tensor_scalar adds (kills 16 rank-1
    bias matmuls).
  - softmax Exp runs on ACT (128 lanes @1.2GHz, ~1.13us/[128,1024] tile) and
    paces the attention loop; every other engine's per-head work is kept
    below it so exp fully hides.
  - Q/K projections are split into [128,512] half-GEMMs accumulating in
    1-bank PSUM tiles and INTERLEAVED two-matmuls-per-kt-step into the
    previous head pair's attention loop: the PE never idles, which also
    keeps it in the 2.4GHz p-state (idle gaps drop it to 1.2GHz).
  - scores computed TRANSPOSED: S^T[k,q] so the attention mask is a
    per-partition bias folded with the scale into the Exp activation.
  - context: ctx[q,0:64] + rowsum at col 64 via lhsT=P^T tile (fp8),
    rhs = V' block [128,65]; two row-blocks share one PSUM bank ([128,130]
    pair tiles); normalize with vector reciprocal + per-partition
    tensor_scalar multiply straight into a [128,256] staging tile; one
    output DMA per (4 heads x row block).
  - input DMAs spread across 4 engine queues (sync/vector/scalar/gpsimd);
    W fp32->bf16 converts round-robin ACT/DVE/Pool.

Built on bacc.Bacc: its compile() legalizes sync waits (1 wait/instruction
hardware limit) via move_matmul_waits_to_ldweights + generate_event_semaphores.
"""

import numpy as np

import concourse.bass as bass
import concourse.bacc as bacc
import concourse.mybir as mybir
import concourse.tile as tile
from concourse.bass_utils import run_bass_kernel_spmd
from concourse.masks import make_identity

F32 = mybir.dt.float32
BF16 = mybir.dt.bfloat16
FP8 = mybir.dt.float8e4

B, S, D, H = 8, 1024, 1024, 16
DH = D // H  # 64
P = 128
NT = S // P  # 8 tiles along any 1024 dim
SC = S // 512  # 2 chunks of 512
SCALE = 1.0 / float(np.sqrt(DH))
N_CORES = 8
VW = DH + 1  # 65: V block width per head (64 cols + ones col)
HG = 4  # heads per output-DMA group

PHASES = 7  # bitmask: 1=x^T, 2=+V proj, 4=+attention loop (profiling aid)


def emit_body(nc, dram, pools):
    (x_d, m_d, wq_d, bq_d, wk_d, bk_d, wv_d, bv_d, o_d) = dram
    (cst, xT_pool, qkT_pool, v_pool, wf_pool, wb_pool, p_pool, small_pool,
     og_pool, ps_big, ps_half, ps_ctx, ident) = pools

    # ---- per-body constants (mask / bias) ----
    mask_cols = cst.tile([P, NT], F32, name="mask_cols", tag="mask_cols")
    nc.sync.dma_start(out=mask_cols, in_=m_d.ap().rearrange("(g p) -> p g", p=P))
    ones_f32 = cst.tile([1, 512], F32, name="ones_f32", tag="ones_f32")
    nc.vector.memset(ones_f32, 1.0)
    ones_row = cst.tile([1, 512], BF16, name="ones_row", tag="ones_row")
    nc.vector.tensor_copy(ones_row, ones_f32)
    # bq/bk as [128, NT] per-partition columns (added in the proj copies)
    b_cols = {}
    for nm, hd in (("bq", bq_d), ("bk", bk_d)):
        t = cst.tile([P, NT], F32, name=f"bcol_{nm}", tag=f"bcol_{nm}")
        nc.sync.dma_start(out=t, in_=hd.ap().rearrange("(g p) -> p g", p=P))
        b_cols[nm] = t
    # bv as a [1, D] bf16 row (rank-1 matmul in the V projection)
    bvf = cst.tile([1, D], F32, name="bvf", tag="bvf")
    nc.sync.dma_start(out=bvf, in_=bv_d.ap().unsqueeze(0))
    bv_row = cst.tile([1, D], BF16, name="bv_row", tag="bv_row")
    nc.vector.tensor_copy(bv_row, bvf)

    if not PHASES & 1:
        return

    # ---- phase 1: X^T via PE transposes (fp32 in, bf16 out via copies
    # split ACT/DVE); W DMAs spread over queues, converts ACT/DVE/Pool ----
    xT = []
    for it in range(NT):
        xT.append(xT_pool.tile([P, S], BF16, name=f"xT{it}", tag=f"xT{it}"))

    w_bf = {}
    w_src = (("wv", wv_d, nc.sync), ("wq", wq_d, nc.scalar),
             ("wk", wk_d, nc.scalar))
    for nm, _, _ in w_src:
        w_bf[nm] = [
            wb_pool.tile([P, D], BF16, name=f"{nm}b{it}", tag=f"{nm}b{it}")
            for it in range(NT)
        ]

    for st in range(NT):
        x_t = wf_pool.tile([P, D], F32, name="x_tile", tag="wf")
        nc.sync.dma_start(out=x_t, in_=x_d.ap()[st * P : (st + 1) * P, :])
        x_bf = wf_pool.tile([P, D], BF16, name="x_bf", tag="xbf")
        nc.gpsimd.tensor_copy(x_bf, x_t)
        for ih in range(NT // 2):
            pt = ps_half.tile([P, 2 * P], BF16, name="pt", tag="half")
            for j in range(2):
                it = 2 * ih + j
                nc.tensor.transpose(
                    pt[:, j * P : (j + 1) * P],
                    x_bf[:, it * P : (it + 1) * P],
                    ident,
                )
            dst0 = xT[2 * ih][:, st * P : (st + 1) * P]
            dst1 = xT[2 * ih + 1][:, st * P : (st + 1) * P]
            if (st + ih) % 2 == 0:
                nc.vector.tensor_copy(dst0, pt[:, 0:P])
                nc.vector.tensor_copy(dst1, pt[:, P : 2 * P])
            else:
                nc.scalar.copy(dst0, pt[:, 0:P])
                nc.scalar.copy(dst1, pt[:, P : 2 * P])

    ci = 0
    for nm, w_d, eng in w_src:
        for it in range(NT):
            wf = wf_pool.tile([P, D], F32, name=f"{nm}f", tag="wf")
            eng.dma_start(out=wf, in_=w_d.ap()[it * P : (it + 1) * P, :])
            conv = (nc.scalar.copy, nc.vector.tensor_copy,
                    nc.gpsimd.tensor_copy)[ci % 3]
            conv(w_bf[nm][it], wf)
            ci += 1

    if not PHASES & 2:
        fin = small_pool.tile([P, DH], F32, name="fin1", tag="bounce")
        nc.vector.tensor_copy(fin, xT[0][:, 0:DH])
        nc.sync.dma_start(out=o_d.ap()[0:P, 0:DH], in_=fin)
        return

    # ---- phase 2: V projection (natural orientation, bf16 matmuls),
    # stored fp8 with x16 scale; ones columns = 16 ----
    v_sb = []
    for st in range(NT):
        v = v_pool.tile([P, H * VW], BF16, name=f"v{st}", tag=f"v{st}")
        nc.gpsimd.memset(v, 1.0)  # ones columns survive at h*65+64
        v_sb.append(v)
    for st in range(NT):
        mm = ps_big.tile([P, S], F32, name="mmv", tag="big")
        for it in range(NT):
            for jc in range(SC):
                nc.tensor.matmul(
                    mm[:, jc * 512 : (jc + 1) * 512],
                    lhsT=xT[it][:, st * P : (st + 1) * P],
                    rhs=w_bf["wv"][it][:, jc * 512 : (jc + 1) * 512],
                    start=(it == 0),
                    stop=False,
                )
        for jc in range(SC):
            nc.tensor.matmul(
                mm[:, jc * 512 : (jc + 1) * 512],
                lhsT=ones_row[0:1, 0:P],
                rhs=bv_row[0:1, jc * 512 : (jc + 1) * 512],
                start=False,
                stop=True,
            )
        dst = v_sb[st].rearrange("p (g c) -> p g c", c=VW)[:, :, 0:DH]
        src = mm.rearrange("p (g c) -> p g c", c=DH)
        nc.vector.tensor_copy(dst, src)

    if not PHASES & 4:
        fin = small_pool.tile([P, DH], F32, name="fin2", tag="bounce")
        nc.vector.tensor_copy(fin, v_sb[0][:, 0:DH])
        nc.sync.dma_start(out=o_d.ap()[0:P, 0:DH], in_=fin)
        return

    # ---- phase 3: attention with interleaved Q/K projection chunks ----
    staging = {}

    def proj_work(jt):
        """Yield ('mm'|'copy', closure) chunks computing q'^T/k'^T[jt] in
        [128,512] half-GEMMs: out fp8 = 16*(sum_i W[i,j] X^T[i,s] + b[j])."""
        for nm, bnm in (("wq", "bq"), ("wk", "bk")):
            dst = qkT_pool.tile([P, S], BF16, name=f"{nm}T{jt}", tag=f"{nm}T")
            if nm == "wq":
                qk = dst
            else:
                kk = dst
            # it-major with both 512-col halves consecutive per weight:
            # the second matmul reuses the loaded stationary tile (skips
            # the serial weight reload).
            mmh = [None, None]

            def mk_mm(it, sc, nm=nm, mmh=mmh):
                def go():
                    if it == 0:
                        mmh[sc] = ps_half.tile(
                            [P, 512], F32, name="mmh", tag="half"
                        )
                    nc.tensor.matmul(
                        mmh[sc],
                        lhsT=w_bf[nm][it][:, jt * P : (jt + 1) * P],
                        rhs=xT[it][:, sc * 512 : (sc + 1) * 512],
                        start=(it == 0),
                        stop=(it == NT - 1),
                    )
                return go

            for it in range(NT):
                for sc in range(SC):
                    yield "mm", mk_mm(it, sc)

            def mk_copy(sc, dst=dst, mmh=mmh, bnm=bnm):
                def go():
                    nc.vector.tensor_scalar_add(
                        dst[:, sc * 512 : (sc + 1) * 512],
                        mmh[sc],
                        b_cols[bnm][:, jt : jt + 1],
                    )
                return go

            for sc in range(SC):
                yield "copy", mk_copy(sc)
        proj_work.out[jt] = (qk, kk)

    proj_work.out = {}

    def drain(chunks, n_mm=None):
        """Emit chunks until n_mm matmuls emitted (None = all)."""
        done = 0
        while chunks and (n_mm is None or done < n_mm):
            kind, go = chunks.pop(0)
            go()
            if kind == "mm":
                done += 1
        # trailing copies ride along with the last requested matmul
        while chunks and chunks[0][0] == "copy" and n_mm is not None:
            chunks.pop(0)[1]()

    def emit_scores_exp_step(h, kt, qT8, kT8):
        ro = (h % 2) * DH
        sps = ps_big.tile([P, S], F32, name="sps", tag="big")
        for qc in range(SC):
            nc.tensor.matmul(
                sps[:, qc * 512 : (qc + 1) * 512],
                lhsT=kT8[ro : ro + DH, kt * P : (kt + 1) * P],
                rhs=qT8[ro : ro + DH, qc * 512 : (qc + 1) * 512],
                start=True,
                stop=True,
            )
        pt = p_pool.tile([P, S], BF16, name="pT", tag="pT")
        nc.scalar.activation(
            pt,
            sps,
            mybir.ActivationFunctionType.Exp,
            bias=mask_cols[:, kt : kt + 1],
            scale=SCALE,
        )
        return pt

    ctx_pair = [None]

    def emit_ctx_qt(h, pT, qt):
        g = h // HG
        if h % HG == 0 and qt == 0:
            for q2 in range(NT):
                staging[q2] = og_pool.tile(
                    [P, HG * DH], F32, name=f"og{q2}", tag=f"og{q2}"
                )
        if qt % 2 == 0:
            ctx_pair[0] = ps_ctx.tile([P, 2 * VW], F32, name="cps", tag="ctx")
        off = (qt % 2) * VW
        cps = ctx_pair[0][:, off : off + VW]
        for kt in range(NT):
            nc.tensor.matmul(
                cps,
                lhsT=pT[kt][:, qt * P : (qt + 1) * P],
                rhs=v_sb[kt][:, h * VW : (h + 1) * VW],
                start=(kt == 0),
                stop=(kt == NT - 1),
            )
        r = small_pool.tile([P, 1], F32, name="recip", tag="recip")
        nc.vector.reciprocal(r, cps[:, DH : DH + 1])
        nc.vector.tensor_scalar_mul(
            staging[qt][:, (h % HG) * DH : (h % HG + 1) * DH], cps[:, 0:DH], r
        )
        if h % HG == HG - 1:
            nc.sync.dma_start(
                out=o_d.ap()[qt * P : (qt + 1) * P, g * HG * DH : (g + 1) * HG * DH],
                in_=staging[qt],
            )

    # proj(0) emitted densely up front
    drain(list(proj_work(0)))

    prev = None  # (h-1, its pT list)
    for h in range(H):
        jt = h // 2
        qT8, kT8 = proj_work.out[jt]
        if h % 2 == 0 and jt + 1 < NT:
            chunks = list(proj_work(jt + 1))
            proj_work.pending = chunks
        pend = getattr(proj_work, "pending", None)
        pT = []
        for kt in range(NT):
            if pend:
                drain(pend, 2)
            if prev is not None:
                emit_ctx_qt(prev[0], prev[1], kt)
            pT.append(emit_scores_exp_step(h, kt, qT8, kT8))
        if h % 2 == 1 and pend:
            drain(pend)  # make sure next jt's proj is complete
        prev = (h, pT)
    for kt in range(NT):
        emit_ctx_qt(prev[0], prev[1], kt)


def build_program(n_reps: int = 1, n_loop: int = 0) -> bass.Bass:
    nc = bacc.Bacc(trn_type="TRN2", target_bir_lowering=False, debug=False)

    x_d = nc.declare_dram_parameter("hidden_states", [S, D], F32, isOutput=False)
    m_d = nc.declare_dram_parameter("attention_mask", [S], F32, isOutput=False)
    wq_d = nc.declare_dram_parameter("Wq", [D, D], F32, isOutput=False)
    bq_d = nc.declare_dram_parameter("bq", [D], F32, isOutput=False)
    wk_d = nc.declare_dram_parameter("Wk", [D, D], F32, isOutput=False)
    bk_d = nc.declare_dram_parameter("bk", [D], F32, isOutput=False)
    wv_d = nc.declare_dram_parameter("Wv", [D, D], F32, isOutput=False)
    bv_d = nc.declare_dram_parameter("bv", [D], F32, isOutput=False)
    o_d = nc.declare_dram_parameter("out", [S, D], F32, isOutput=True)
    dram = (x_d, m_d, wq_d, bq_d, wk_d, bk_d, wv_d, bv_d, o_d)

    with tile.TileContext(nc) as tc:
        with (
            tc.tile_pool(name="consts", bufs=1) as cst,
            tc.tile_pool(name="xT", bufs=1) as xT_pool,
            tc.tile_pool(name="qkT", bufs=2) as qkT_pool,
            tc.tile_pool(name="vsb", bufs=1) as v_pool,
            tc.tile_pool(name="wf", bufs=4) as wf_pool,
            tc.tile_pool(name="wb", bufs=1) as wb_pool,
            tc.tile_pool(name="pT", bufs=16) as p_pool,
            tc.tile_pool(name="small", bufs=16) as small_pool,
            tc.tile_pool(name="og", bufs=2) as og_pool,
            # PSUM: scores/V [128,1024] 2x2 banks, proj halves + transposes
            # [128,512] 3x1 banks, ctx pairs [128,130] 1x1 bank -> 8 banks.
            tc.tile_pool(name="psbig", bufs=2, space="PSUM") as ps_big,
            tc.tile_pool(name="pshalf", bufs=3, space="PSUM") as ps_half,
            tc.tile_pool(name="psctx", bufs=1, space="PSUM") as ps_ctx,
        ):
            ident = cst.tile([P, P], BF16, name="ident", tag="ident")
            make_identity(nc, ident)
            pools = (cst, xT_pool, qkT_pool, v_pool, wf_pool, wb_pool, p_pool,
                     small_pool, og_pool, ps_big, ps_half, ps_ctx, ident)
            if n_loop:
                with tc.For_i(0, n_loop, 1):
                    emit_body(nc, dram, pools)
            else:
                for _ in range(n_reps):
                    emit_body(nc, dram, pools)
    nc.compile()
    return nc


_NC_CACHE = None


def _get_nc():
    global _NC_CACHE
    if _NC_CACHE is None:
        _NC_CACHE = build_program()
    return _NC_CACHE


def make_in_maps(hidden_states, attention_mask, Wq, bq, Wk, bk, Wv, bv):
    hs = np.ascontiguousarray(np.asarray(hidden_states, dtype=np.float32))
    am = np.ascontiguousarray(
        np.asarray(attention_mask, dtype=np.float32).reshape(B, S)
    )
    shared = {
        "Wq": np.ascontiguousarray(np.asarray(Wq, dtype=np.float32)),
        "bq": np.ascontiguousarray(np.asarray(bq, dtype=np.float32)),
        "Wk": np.ascontiguousarray(np.asarray(Wk, dtype=np.float32)),
        "bk": np.ascontiguousarray(np.asarray(bk, dtype=np.float32)),
        "Wv": np.ascontiguousarray(np.asarray(Wv, dtype=np.float32)),
        "bv": np.ascontiguousarray(np.asarray(bv, dtype=np.float32)),
    }
    return [
        {"hidden_states": hs[b], "attention_mask": am[b], **shared}
        for b in range(B)
    ]


def kernel(hidden_states, attention_mask, Wq, bq, Wk, bk, Wv, bv):
    nc = _get_nc()
    in_maps = make_in_maps(hidden_states, attention_mask, Wq, bq, Wk, bk, Wv, bv)
    res = run_bass_kernel_spmd(nc, in_maps, list(range(N_CORES))).results
    out = np.stack([np.asarray(res[b]["out"], dtype=np.float32) for b in range(B)])
    return out
